# revision 1
# baseline (speedup 1.0000x reference)
"""Trainium2 Bass kernel for structured-sparse matmul.

Computes: out[b,s,o] = sum_k x[b,s,sparse_idx[k]] * sparse_values[o,k]
  x: [4, 2048, 4096] f32, sparse_values: [4096, 1024] f32,
  sparse_idx: [1024] int64 (sorted, unique) -> out [4, 2048, 4096] f32

Strategy (8 NeuronCores, data-parallel over rows m = b*s):
  Per core (M=1024 rows):
    Phase A (on device): PE-transpose x tiles (regular f32r matmul against
      an identity), then gather n->k via matmul with one-hot selection
      blocks G (built on the host from sparse_idx, which is compile-time
      metadata): x_gT[k, m].
    Phase B: GEMM out[m, o] = x_gT.T @ W^T[k, o] with float32r
      (full-rate fp22 multiplies, fp32 accumulate).
  Host only slices x, transposes sparse_values (weight layout prep),
  and expands sparse_idx into the tiny selection blocks.

Measured (neuron-profile, core 0): 208.6 us HW exec with x chunks alternated
across the sync/scalar HWDGE rings (vs 213-225 us, median ~218, single-ring;
alternation cuts head-of-line blocking in the in-order x stream). PE ~185 us
busy; the 512 GEMM matmuls run at ~222 ns per [128x128x512] f32r MM
(stream-rate, 91% PE occupancy). Remaining span = ~10 us startup dead zone +
~12 us Tile drain tail + x-delivery jitter.
Relative error vs the fp32 reference: 1.47e-4 (fp22 multiply truncation).
"""

import sys

if "/opt/trn_rl_repo" not in sys.path:
    sys.path.insert(0, "/opt/trn_rl_repo")

import numpy as np

B, S, N_IN = 4, 2048, 4096
N_OUT, N_SPARSE = 4096, 1024
N_CORES = 8
M_TOT = B * S            # 8192
M = M_TOT // N_CORES     # 1024 rows per core
P = 128
NKT = N_SPARSE // P      # 8 k-tiles
NNB = N_IN // P          # 32 n-blocks
N_MT = M // P            # 8 m-tiles per core
MB = 512                 # m-batch for transpose/gather staging
N_BATCH = M // MB        # 2
MSUB = MB // P           # 4 m-subtiles per batch
NQ = 4                   # x streamed in quarter-width column chunks
NQW = N_IN // NQ         # 1024 columns per chunk
O_TILE = 512
NOS = N_OUT // O_TILE    # 8 o-slices

_cache: dict = {}


def _build_gather_blocks(idx: np.ndarray):
    """Expand sparse_idx into one-hot selection blocks.

    For k-tile kt and n-block b, G[n, krel] = 1 iff idx[kt*128+krel] == b*128+n.
    Returns (g_all [NB,128,128] f32, blocks_per_kt: list of lists of (bi, b)).
    """
    mats = []
    blocks_per_kt = []
    for kt in range(NKT):
        ks = idx[kt * P:(kt + 1) * P]
        bs = sorted(set(int(k) // P for k in ks))
        entries = []
        for b in bs:
            mat = np.zeros((P, P), dtype=np.float32)
            for krel, k in enumerate(ks):
                if int(k) // P == b:
                    mat[int(k) % P, krel] = 1.0
            entries.append((len(mats), b))
            mats.append(mat)
        blocks_per_kt.append(entries)
    return np.stack(mats), blocks_per_kt


def _build_nc(blocks_per_kt, nb_total):
    import concourse.mybir as mybir
    import concourse.tile as tile
    from concourse import bacc

    F32R = mybir.dt.float32r
    F32 = mybir.dt.float32

    nc = bacc.Bacc("TRN2", target_bir_lowering=False, debug=False)
    x = nc.dram_tensor("x", [M, N_IN], F32R, kind="ExternalInput")
    wt = nc.dram_tensor("wt", [NOS, P, NKT, O_TILE], F32R, kind="ExternalInput")
    g = nc.dram_tensor("g", [P, nb_total, P], F32R, kind="ExternalInput")
    ident = nc.dram_tensor("ident", [P, P], F32R, kind="ExternalInput")
    out = nc.dram_tensor("out", [M, N_OUT], F32, kind="ExternalOutput")

    with tile.TileContext(nc) as tc:
        with (
            tc.tile_pool(name="const", bufs=1) as const_pool,
            tc.tile_pool(name="gpool", bufs=1) as g_pool,
            tc.tile_pool(name="xgpool", bufs=1) as xg_pool,
            tc.tile_pool(name="xin", bufs=2) as x_pool,
            tc.tile_pool(name="xtpool", bufs=1) as xt_pool,
            tc.tile_pool(name="wpool", bufs=2) as wt_pool,
            tc.tile_pool(name="opool", bufs=4) as o_pool,
            tc.tile_pool(name="ps_t", bufs=3, space="PSUM") as pst,
            tc.tile_pool(name="ps_g", bufs=2, space="PSUM") as psg,
            tc.tile_pool(name="ps_b", bufs=3, space="PSUM") as psb,
        ):
            ident_sb = const_pool.tile([P, P], F32R)
            nc.sync.dma_start(ident_sb[:], ident[:])
            # x_gT resident: [k-part, kt, m]
            xg_sb = xg_pool.tile([P, NKT, M], F32R)

            # PE warm-up: the HAM clock gate keeps the PE at 1.2 GHz until it
            # sees ~3.4us of sustained activity. Burn dummy matmuls while the
            # first x tile is still in flight so the real work runs at 2.4.
            for w in range(22):
                wps = psb.tile([P, O_TILE], F32, tag="psb", name=f"warm{w}")
                nc.tensor.matmul(
                    wps[:, :P], ident_sb[:], ident_sb[:], start=True, stop=True
                )

            # wt slices are prefetched on the scalar (ACT) HWDGE ring so they
            # don't queue behind the x loads on the sync ring.
            wt_tiles = {}

            def ensure_wt(s):
                if s >= NOS or s in wt_tiles:
                    return
                t = wt_pool.tile([P, NKT, O_TILE], F32R, tag="wt", name=f"wt{s}")
                nc.scalar.dma_start(t[:, :NKT // 2, :], wt[s, :, :NKT // 2, :])
                nc.scalar.dma_start(t[:, NKT // 2:, :], wt[s, :, NKT // 2:, :])
                wt_tiles[s] = t

            def emit_b(s, t_range, prefetch_at=None, prefetch_s=None):
                wt_sb = wt_tiles[s]
                for t in t_range:
                    ps = psb.tile([P, O_TILE], F32, tag="psb",
                                  name=f"psb{s}_{t}")
                    for kt in range(NKT):
                        nc.tensor.matmul(
                            ps[:],
                            xg_sb[:, kt, t * P:(t + 1) * P],
                            wt_sb[:, kt, :],
                            start=(kt == 0),
                            stop=(kt == NKT - 1),
                        )
                    o_sb = o_pool.tile([P, O_TILE], F32, tag="ob",
                                       name=f"ob{s}_{t}")
                    # DVE is idle during the GEMM phase; keep ACT free for
                    # the wt prefetch DMAs.
                    nc.vector.tensor_copy(o_sb[:], ps[:])
                    nc.sync.dma_start(
                        out[t * P:(t + 1) * P, s * O_TILE:(s + 1) * O_TILE],
                        o_sb[:],
                    )
                    if prefetch_at is not None and t == prefetch_at:
                        ensure_wt(prefetch_s)

            # ---- Phase A: transpose + gather ----
            g_sb = None
            for batch in range(N_BATCH):
                m0 = batch * MB
                xt_sb = xt_pool.tile([P, NNB, MB], F32R, tag="xt")

                def emit_gather(kt):
                    entries = blocks_per_kt[kt]
                    ps = psg.tile([P, MB], F32, tag="psg", name=f"psg{batch}_{kt}")
                    for i, (bi, b) in enumerate(entries):
                        nc.tensor.matmul(
                            ps[:],
                            g_sb[:, bi, :],
                            xt_sb[:, b, :],
                            start=(i == 0),
                            stop=(i == len(entries) - 1),
                        )
                    nc.scalar.copy(xg_sb[:, kt, m0:m0 + MB], ps[:])

                gathered = set()
                NBQ = NNB // NQ  # n-blocks per chunk (8)
                for q in range(NQ):
                    # x streamed as [128 part(m), MSUB, 1024 cols] quarter
                    # tiles, loaded by per-j 512KB DMAs so transposes start
                    # as soon as one m-subtile lands.
                    x_sb = x_pool.tile([P, MSUB, NQW], F32R, tag="xin")
                    for j in range(MSUB):
                        eng = nc.sync if j % 2 == 0 else nc.scalar
                        eng.dma_start(
                            x_sb[:, j, :],
                            x[m0 + j * P:m0 + (j + 1) * P,
                              q * NQW:(q + 1) * NQW],
                        )
                    if batch == 0 and q == 1:
                        # g lands behind q0/q1's scalar-ring chunks, just
                        # ahead of the first gathers (~25us in).
                        g_sb = g_pool.tile([P, nb_total, P], F32R)
                        nc.scalar.dma_start(g_sb[:], g[:])
                        ensure_wt(0)
                        ensure_wt(1)
                    for j in range(MSUB):
                        for half in range(2):
                            ps = pst.tile([P, NBQ // 2, P], F32, tag="pst")
                            for nbq in range(NBQ // 2):
                                c = half * (NBQ // 2) + nbq
                                nc.tensor.matmul(
                                    ps[:, nbq, :],
                                    x_sb[:, j, c * P:(c + 1) * P],
                                    ident_sb[:],
                                    start=True,
                                    stop=True,
                                )
                            nb0 = q * NBQ + half * (NBQ // 2)
                            dst = xt_sb[:, nb0:nb0 + NBQ // 2,
                                        j * P:(j + 1) * P]
                            # Alternate eviction engine so neither DVE nor
                            # ACT gates PSUM recycling.
                            if (j + half) % 2 == 0:
                                nc.vector.tensor_copy(dst, ps[:])
                            else:
                                nc.scalar.copy(dst, ps[:])
                    # Emit every gather whose source n-blocks are now all
                    # transposed — keeps the PE stream dense and spreads the
                    # gather work across the batch instead of bunching it.
                    nb_done = (q + 1) * (NNB // NQ)
                    for kt in range(NKT):
                        if kt in gathered or g_sb is None:
                            continue
                        if all(b < nb_done for _, b in blocks_per_kt[kt]):
                            emit_gather(kt)
                            gathered.add(kt)
                for kt in range(NKT):
                    if kt not in gathered:
                        emit_gather(kt)

            # ---- Phase B: main GEMM ----
            for s in range(NOS):
                ensure_wt(s)
                emit_b(s, range(N_MT), prefetch_at=2, prefetch_s=s + 2)
    nc.compile()
    return nc


def _get_compiled(idx: np.ndarray):
    key = idx.tobytes()
    if key not in _cache:
        g_all, blocks_per_kt = _build_gather_blocks(idx)
        nc = _build_nc(blocks_per_kt, g_all.shape[0])
        _cache[key] = (nc, g_all)
    return _cache[key]


def _run(inputs, trace=False, trace_kwargs=None):
    from concourse.bass_utils import run_bass_kernel_spmd

    x = np.ascontiguousarray(np.asarray(inputs["x"], dtype=np.float32))
    sv = np.asarray(inputs["sparse_values"], dtype=np.float32)
    idx = np.asarray(inputs["sparse_idx"]).astype(np.int64)

    nc, g_all = _get_compiled(idx)

    x2 = x.reshape(M_TOT, N_IN)
    # wt swizzled for contiguous per-partition DMA: [o-slice, k%128, k//128, o]
    wtv = np.ascontiguousarray(
        sv.T.reshape(NKT, P, NOS, O_TILE).transpose(2, 1, 0, 3)
    )
    # g swizzled to [n-rel (partition), block, k-rel]
    g_swz = np.ascontiguousarray(g_all.transpose(1, 0, 2))
    in_maps = [
        {
            "x": np.ascontiguousarray(x2[c * M:(c + 1) * M]),
            "wt": wtv,
            "g": g_swz,
            "ident": np.eye(P, dtype=np.float32),
        }
        for c in range(N_CORES)
    ]
    res = run_bass_kernel_spmd(
        nc,
        in_maps,
        core_ids=list(range(N_CORES)),
        trace=trace,
        **(trace_kwargs or {}),
    )
    full = np.concatenate([r["out"] for r in res.results], axis=0)
    return full.reshape(B, S, N_OUT), res


def kernel(**inputs) -> np.ndarray:
    out, _ = _run(inputs)
    return out



# revision 5
# speedup vs baseline: 1.0499x; 1.0499x over previous
"""Trainium2 Bass kernel for structured-sparse matmul.

Computes: out[b,s,o] = sum_k x[b,s,sparse_idx[k]] * sparse_values[o,k]
  x: [4, 2048, 4096] f32, sparse_values: [4096, 1024] f32,
  sparse_idx: [1024] int64 (sorted, unique) -> out [4, 2048, 4096] f32

Strategy (8 NeuronCores, data-parallel over rows m = b*s, M=1024/core):
  Pipelined 256-row units. Per unit:
    1. x chunk DMAs (f32, alternating sync/scalar rings)
    2. PE transposes in is_transpose mode (f32r, 1.5 cy/row vs the 4 cy/row
       a plain 128-wide f32r matmul costs), PSUM evicted with a fused cast
       to bf16 xt.
    3. Gather n->k via one-hot blocks G (bf16): PE matmuls, stationary G
       loads at 128 cy (bf16) so a 256-wide moving operand hides them.
       Eviction casts to bf16 xg.
    4. GEMM out[m,o] = xg^T @ W^T in bf16 (W host-cast to bf16, halving
       its DMA), 512-wide moving, f32 PSUM, f32 output.
  Unit u+1's x stream overlaps unit u's GEMM; W/g prefetch on both rings.
  PE warm-up transposes bridge the first x DMA so the HAM clock gate is
  released (~3.4us sustained) before real work.
"""

import sys

if "/opt/trn_rl_repo" not in sys.path:
    sys.path.insert(0, "/opt/trn_rl_repo")

import ml_dtypes
import numpy as np

B, S, N_IN = 4, 2048, 4096
N_OUT, N_SPARSE = 4096, 1024
N_CORES = 8
M_TOT = B * S            # 8192
M = M_TOT // N_CORES     # 1024 rows per core
P = 128
NKT = N_SPARSE // P      # 8 k-tiles
NNB = N_IN // P          # 32 n-blocks
UNIT = 256               # pipeline unit (rows)
N_UNITS = M // UNIT      # 4
MSUB = UNIT // P         # 2 m-subtiles per unit
O_TILE = 512
NOS = N_OUT // O_TILE    # 8 o-slices
NH = 2                   # x column halves per unit
HBLK = NNB // NH         # 16 n-blocks per half
N_WARM = 32

_cache: dict = {}


def _build_gather_blocks(idx: np.ndarray):
    """Expand sparse_idx into one-hot selection blocks.

    For k-tile kt and n-block b, G[n, krel] = 1 iff idx[kt*128+krel] == b*128+n.
    Returns (g_all [NB,128,128] f32, blocks_per_kt: list of lists of (bi, b)).
    """
    mats = []
    blocks_per_kt = []
    for kt in range(NKT):
        ks = idx[kt * P:(kt + 1) * P]
        bs = sorted(set(int(k) // P for k in ks))
        entries = []
        for b in bs:
            mat = np.zeros((P, P), dtype=np.float32)
            for krel, k in enumerate(ks):
                if int(k) // P == b:
                    mat[int(k) % P, krel] = 1.0
            entries.append((len(mats), b))
            mats.append(mat)
        blocks_per_kt.append(entries)
    return np.stack(mats), blocks_per_kt


def _build_nc(blocks_per_kt, nb_total):
    import concourse.mybir as mybir
    import concourse.tile as tile
    from concourse import bacc

    F32R = mybir.dt.float32r
    F32 = mybir.dt.float32
    BF16 = mybir.dt.bfloat16

    nc = bacc.Bacc("TRN2", target_bir_lowering=False, debug=False)
    x = nc.dram_tensor("x", [M, N_IN], F32R, kind="ExternalInput")
    wt = nc.dram_tensor("wt", [P, NOS, NKT, O_TILE], BF16, kind="ExternalInput")
    g = nc.dram_tensor("g", [P, nb_total, P], BF16, kind="ExternalInput")
    ident = nc.dram_tensor("ident", [P, P], F32R, kind="ExternalInput")
    out = nc.dram_tensor("out", [M, N_OUT], F32, kind="ExternalOutput")

    # blocks of kt fully contained in column half 0 (for progressive gather)
    kt_ready_h = []
    for kt in range(NKT):
        entries = blocks_per_kt[kt]
        kt_ready_h.append(0 if all(b < HBLK for _, b in entries) else 1)

    with tile.TileContext(nc) as tc:
        with (
            tc.tile_pool(name="const", bufs=1) as const_pool,
            tc.tile_pool(name="gpool", bufs=1) as g_pool,
            tc.tile_pool(name="wpool", bufs=1) as wt_pool,
            tc.tile_pool(name="xin", bufs=2) as x_pool,
            tc.tile_pool(name="xtpool", bufs=1) as xt_pool,
            tc.tile_pool(name="xgpool", bufs=2) as xg_pool,
            tc.tile_pool(name="opool", bufs=4) as o_pool,
            tc.tile_pool(name="ps_t", bufs=2, space="PSUM") as pst,
            tc.tile_pool(name="ps_g", bufs=2, space="PSUM") as psg,
            tc.tile_pool(name="ps_b", bufs=3, space="PSUM") as psb,
        ):
            ident_sb = const_pool.tile([P, P], F32R)
            nc.sync.dma_start(ident_sb[:], ident[:])
            g_sb = g_pool.tile([P, nb_total, P], BF16)
            wt_sb = wt_pool.tile([P, NOS, NKT, O_TILE], BF16)
            xt_sb = xt_pool.tile([P, NNB, UNIT], BF16)

            # PE warm-up: release the HAM clock gate (needs ~3.4us sustained
            # activity) while the first x chunks are in flight.
            for w in range(N_WARM):
                wps = pst.tile([P, 4, P], F32R, tag="pst", name=f"warm{w}")
                nc.tensor.transpose(wps[:, 0, :], ident_sb[:], ident_sb[:])

            wt_loaded = set()

            def ensure_wt(s, eng):
                if s >= NOS or s in wt_loaded:
                    return
                eng.dma_start(wt_sb[:, s, :, :], wt[:, s, :, :])
                wt_loaded.add(s)

            ev_flip = [0]

            def evict(dst, src):
                # Alternate PSUM-eviction engine so neither DVE nor ACT
                # becomes the serial bottleneck.
                if ev_flip[0] % 2 == 0:
                    nc.vector.tensor_copy(dst, src)
                else:
                    nc.scalar.copy(dst, src)
                ev_flip[0] += 1

            def dma_x(u):
                # x chunk DMAs: (half, j) -> 4 x 1MB, h0 before h1 so early
                # gathers can fire; rings alternate per j. g halves slot in
                # behind unit-0's matching x chunks.
                m0 = u * UNIT
                t_ = x_pool.tile([P, MSUB, N_IN], F32R, tag="xin")
                for h in range(NH):
                    for j in range(MSUB):
                        eng = nc.sync if j % 2 == 0 else nc.scalar
                        eng.dma_start(
                            t_[:, j, h * (N_IN // NH):(h + 1) * (N_IN // NH)],
                            x[m0 + j * P:m0 + (j + 1) * P,
                              h * (N_IN // NH):(h + 1) * (N_IN // NH)],
                        )
                    if u == 0:
                        nb_h = sum(
                            len(blocks_per_kt[kt]) for kt in range(NKT)
                            if kt_ready_h[kt] == 0
                        )
                        # g is ordered so half-0-ready kts' blocks come first
                        if h == 0:
                            nc.scalar.dma_start(
                                g_sb[:, :nb_h, :], g[:, :nb_h, :])
                        else:
                            nc.scalar.dma_start(
                                g_sb[:, nb_h:, :], g[:, nb_h:, :])
                            ensure_wt(0, nc.sync)
                            ensure_wt(1, nc.scalar)
                return t_

            x_sb = dma_x(0)
            for u in range(N_UNITS):
                m0 = u * UNIT
                # ---- transpose + gather (progressive over column halves) ----
                gathered = set()

                def emit_gather(kt, u=u):
                    entries = blocks_per_kt[kt]
                    ps = psg.tile([P, UNIT], F32, tag="psg",
                                  name=f"psg{u}_{kt}")
                    for i, (bi, b) in enumerate(entries):
                        nc.tensor.matmul(
                            ps[:],
                            g_sb[:, bi, :],
                            xt_sb[:, b, :],
                            start=(i == 0),
                            stop=(i == len(entries) - 1),
                        )
                    evict(xg_sb[:, kt, :], ps[:])

                xg_sb = xg_pool.tile([P, NKT, UNIT], BF16, tag="xg")
                for h in range(NH):
                    for j in range(MSUB):
                        for cc in range(HBLK // 4):
                            ps = pst.tile([P, 4, P], F32R, tag="pst")
                            for q in range(4):
                                blk = h * HBLK + cc * 4 + q
                                nc.tensor.transpose(
                                    ps[:, q, :],
                                    x_sb[:, j, blk * P:(blk + 1) * P],
                                    ident_sb[:],
                                )
                            blk0 = h * HBLK + cc * 4
                            evict(
                                xt_sb[:, blk0:blk0 + 4, j * P:(j + 1) * P],
                                ps[:],
                            )
                    for kt in range(NKT):
                        if kt not in gathered and kt_ready_h[kt] <= h:
                            emit_gather(kt)
                            gathered.add(kt)

                # Prefetch: next unit's x + remaining W ride the rings while
                # this unit's GEMM keeps the PE busy.
                x_next = dma_x(u + 1) if u + 1 < N_UNITS else None
                if u == 0:
                    for s in range(2, NOS):
                        ensure_wt(s, nc.sync if s % 2 == 0 else nc.scalar)

                # ---- GEMM for this unit ----
                for s in range(NOS):
                    for t in range(MSUB):
                        ps = psb.tile([P, O_TILE], F32, tag="psb",
                                      name=f"psb{u}_{s}_{t}")
                        for kt in range(NKT):
                            nc.tensor.matmul(
                                ps[:],
                                xg_sb[:, kt, t * P:(t + 1) * P],
                                wt_sb[:, s, kt, :],
                                start=(kt == 0),
                                stop=(kt == NKT - 1),
                            )
                        o_sb = o_pool.tile([P, O_TILE], F32, tag="ob",
                                           name=f"ob{u}_{s}_{t}")
                        evict(o_sb[:], ps[:])
                        eng = nc.sync if (s + t) % 2 == 0 else nc.scalar
                        eng.dma_start(
                            out[m0 + t * P:m0 + (t + 1) * P,
                                s * O_TILE:(s + 1) * O_TILE],
                            o_sb[:],
                        )
                if x_next is not None:
                    x_sb = x_next
    nc.compile()
    return nc


def _reorder_g(g_all, blocks_per_kt):
    """Order g blocks so kts gatherable from column half 0 come first."""
    order = []
    new_blocks = []
    for pass_h in range(2):
        for kt in range(NKT):
            entries = blocks_per_kt[kt]
            ready = 0 if all(b < HBLK for _, b in entries) else 1
            if ready != pass_h:
                continue
            new_entries = []
            for bi, b in entries:
                new_entries.append((len(order), b))
                order.append(bi)
            new_blocks.append((kt, new_entries))
    new_blocks.sort(key=lambda e: e[0])
    return g_all[order], [e for _, e in new_blocks]


def _get_compiled(idx: np.ndarray):
    key = idx.tobytes()
    if key not in _cache:
        g_all, blocks_per_kt = _build_gather_blocks(idx)
        g_all, blocks_per_kt = _reorder_g(g_all, blocks_per_kt)
        nc = _build_nc(blocks_per_kt, g_all.shape[0])
        _cache[key] = (nc, g_all)
    return _cache[key]


def _run(inputs, trace=False, trace_kwargs=None):
    from concourse.bass_utils import run_bass_kernel_spmd

    x = np.ascontiguousarray(np.asarray(inputs["x"], dtype=np.float32))
    sv = np.asarray(inputs["sparse_values"], dtype=np.float32)
    idx = np.asarray(inputs["sparse_idx"]).astype(np.int64)

    nc, g_all = _get_compiled(idx)

    x2 = x.reshape(M_TOT, N_IN)
    # wt swizzled for per-partition DMA: [k%128, o-slice, k//128, o], bf16
    wtv = np.ascontiguousarray(
        sv.T.reshape(NKT, P, NOS, O_TILE).transpose(1, 2, 0, 3)
    ).astype(ml_dtypes.bfloat16)
    # g swizzled to [n-rel (partition), block, k-rel], bf16 (one-hot: exact)
    g_swz = np.ascontiguousarray(
        g_all.transpose(1, 0, 2)).astype(ml_dtypes.bfloat16)
    in_maps = [
        {
            "x": np.ascontiguousarray(x2[c * M:(c + 1) * M]),
            "wt": wtv,
            "g": g_swz,
            "ident": np.eye(P, dtype=np.float32),
        }
        for c in range(N_CORES)
    ]
    res = run_bass_kernel_spmd(
        nc,
        in_maps,
        core_ids=list(range(N_CORES)),
        trace=trace,
        **(trace_kwargs or {}),
    )
    full = np.concatenate([r["out"] for r in res.results], axis=0)
    return full.reshape(B, S, N_OUT), res


def kernel(**inputs) -> np.ndarray:
    out, _ = _run(inputs)
    return out


# revision 8
# speedup vs baseline: 1.0721x; 1.0212x over previous
"""Trainium2 Bass kernel for structured-sparse matmul.

Computes: out[b,s,o] = sum_k x[b,s,sparse_idx[k]] * sparse_values[o,k]
  x: [4, 2048, 4096] f32, sparse_values: [4096, 1024] f32,
  sparse_idx: [1024] int64 (sorted, unique) -> out [4, 2048, 4096] f32

Strategy (8 NeuronCores, data-parallel over rows m = b*s, M=1024/core):
  Pipelined 256-row units: per unit, PE-transpose x (is_transpose mode,
  f32r 1.5 cy/row), gather n->k with one-hot blocks (bf16), then GEMM in
  bf16 (W host-cast, halving its DMA).  Engine discipline learned from
  the v2 trace:
    - DVE owns every PSUM eviction; the Sync/Scalar queues carry ONLY
      HWDGE DMA triggers.  (Mixing ACT evictions with DMA triggers
      head-of-line blocked the PE for 5-8us per unit, re-tripping the
      HAM clock gate and running whole GEMM units at 1.2 GHz.)
    - x streams as 1MB chunks through an 8-buffer pool: ~2 units of
      lookahead, paced automatically by the pool's WAR dependencies.
    - Warm-up transposes read a memset scratch tile, not a DMA'd one,
      so the PE is busy from ~1.5us and the HAM gate opens before the
      first real transpose.
"""

import sys

if "/opt/trn_rl_repo" not in sys.path:
    sys.path.insert(0, "/opt/trn_rl_repo")

import ml_dtypes
import numpy as np

B, S, N_IN = 4, 2048, 4096
N_OUT, N_SPARSE = 4096, 1024
N_CORES = 8
M_TOT = B * S            # 8192
M = M_TOT // N_CORES     # 1024 rows per core
P = 128
NKT = N_SPARSE // P      # 8 k-tiles
NNB = N_IN // P          # 32 n-blocks
UNIT = 256               # pipeline unit (rows)
N_UNITS = M // UNIT      # 4
MSUB = UNIT // P         # 2 m-subtiles per unit
O_TILE = 512
NOS = N_OUT // O_TILE    # 8 o-slices
NH = 2                   # x column halves per unit
HBLK = NNB // NH         # 16 n-blocks per half
XCW = N_IN // NH         # 2048 cols per x chunk
N_WARM = 56

_cache: dict = {}


def _build_gather_blocks(idx: np.ndarray):
    """Expand sparse_idx into one-hot selection blocks.

    For k-tile kt and n-block b, G[n, krel] = 1 iff idx[kt*128+krel] == b*128+n.
    Returns (g_all [NB,128,128] f32, blocks_per_kt: list of lists of (bi, b)).
    """
    mats = []
    blocks_per_kt = []
    for kt in range(NKT):
        ks = idx[kt * P:(kt + 1) * P]
        bs = sorted(set(int(k) // P for k in ks))
        entries = []
        for b in bs:
            mat = np.zeros((P, P), dtype=np.float32)
            for krel, k in enumerate(ks):
                if int(k) // P == b:
                    mat[int(k) % P, krel] = 1.0
            entries.append((len(mats), b))
            mats.append(mat)
        blocks_per_kt.append(entries)
    return np.stack(mats), blocks_per_kt


def _build_nc(blocks_per_kt, nb_total):
    import concourse.mybir as mybir
    import concourse.tile as tile
    from concourse import bacc

    F32R = mybir.dt.float32r
    F32 = mybir.dt.float32
    BF16 = mybir.dt.bfloat16

    nc = bacc.Bacc("TRN2", target_bir_lowering=False, debug=False)
    x = nc.dram_tensor("x", [M, N_IN], F32R, kind="ExternalInput")
    wt = nc.dram_tensor("wt", [P, NOS, NKT, O_TILE], BF16, kind="ExternalInput")
    g = nc.dram_tensor("g", [P, nb_total, P], BF16, kind="ExternalInput")
    ident = nc.dram_tensor("ident", [P, P], F32R, kind="ExternalInput")
    out = nc.dram_tensor("out", [M, N_OUT], F32, kind="ExternalOutput")

    # earliest column-half after which kt's gather can run
    kt_ready_h = []
    for kt in range(NKT):
        entries = blocks_per_kt[kt]
        kt_ready_h.append(0 if all(b < HBLK for _, b in entries) else 1)

    with tile.TileContext(nc) as tc:
        with (
            tc.tile_pool(name="const", bufs=1) as const_pool,
            tc.tile_pool(name="gpool", bufs=1) as g_pool,
            tc.tile_pool(name="wpool", bufs=1) as wt_pool,
            tc.tile_pool(name="xch", bufs=8) as x_pool,
            tc.tile_pool(name="xtpool", bufs=1) as xt_pool,
            tc.tile_pool(name="xgpool", bufs=2) as xg_pool,
            tc.tile_pool(name="opool", bufs=4) as o_pool,
            tc.tile_pool(name="ps_t", bufs=2, space="PSUM") as pst,
            tc.tile_pool(name="ps_g", bufs=2, space="PSUM") as psg,
            tc.tile_pool(name="ps_b", bufs=3, space="PSUM") as psb,
        ):
            # Warm-up on a memset scratch tile: no DMA dependency, so the
            # PE starts within ~1.5us of kernel start and the HAM clock
            # gate is released before real work arrives.  (bf16: an f32r
            # memset fails the walrus memset_set_value_type ISA check.)
            scratch = const_pool.tile([P, P], BF16, name="scratch")
            nc.vector.memset(scratch[:], 0.0)
            for w in range(N_WARM):
                wps = psg.tile([P, UNIT], F32, tag="psg", name=f"warm{w}")
                nc.tensor.matmul(
                    wps[:, :P], scratch[:], scratch[:], start=True, stop=True
                )

            ident_sb = const_pool.tile([P, P], F32R)
            nc.sync.dma_start(ident_sb[:], ident[:])
            g_sb = g_pool.tile([P, nb_total, P], BF16)
            wt_sb = wt_pool.tile([P, NOS, NKT, O_TILE], BF16)
            xt_sb = xt_pool.tile([P, NNB, UNIT], BF16)

            ring_flip = [0]

            def ring():
                ring_flip[0] += 1
                return nc.sync if ring_flip[0] % 2 == 0 else nc.scalar

            def evict(dst, src):
                # DVE owns all PSUM evictions (GPSIMD has no PSUM port;
                # ACT must stay trigger-only to avoid head-of-line blocks).
                nc.vector.tensor_copy(dst, src)

            def dma_chunks(u):
                # 4 x 1MB chunk DMAs for unit u, h0 before h1; the 8-buf
                # pool paces triggers ~2 units ahead of consumption.
                tiles = []
                m0 = u * UNIT
                for h in range(NH):
                    for j in range(MSUB):
                        t_ = x_pool.tile([P, XCW], F32R, tag="xch",
                                         name=f"x{u}_{h}_{j}")
                        ring().dma_start(
                            t_[:],
                            x[m0 + j * P:m0 + (j + 1) * P,
                              h * XCW:(h + 1) * XCW],
                        )
                        tiles.append(t_)
                return tiles

            wt_loaded = set()

            def ensure_wt(s):
                if s >= NOS or s in wt_loaded:
                    return
                ring().dma_start(wt_sb[:, s, :, :], wt[:, s, :, :])
                wt_loaded.add(s)

            nb_h = sum(
                len(blocks_per_kt[kt]) for kt in range(NKT)
                if kt_ready_h[kt] == 0
            )
            chunks = {0: dma_chunks(0)}
            # g halves + first W slices ride behind unit-0's x chunks.
            nc.scalar.dma_start(g_sb[:, :nb_h, :], g[:, :nb_h, :])
            nc.sync.dma_start(g_sb[:, nb_h:, :], g[:, nb_h:, :])
            ensure_wt(0)
            ensure_wt(1)
            chunks[1] = dma_chunks(1)

            for u in range(N_UNITS):
                m0 = u * UNIT
                gathered = set()

                def emit_gather(kt, u=u, xg_sb=None):
                    entries = blocks_per_kt[kt]
                    ps = psg.tile([P, UNIT], F32, tag="psg",
                                  name=f"psg{u}_{kt}")
                    for i, (bi, b) in enumerate(entries):
                        nc.tensor.matmul(
                            ps[:],
                            g_sb[:, bi, :],
                            xt_sb[:, b, :],
                            start=(i == 0),
                            stop=(i == len(entries) - 1),
                        )
                    evict(xg_sb[:, kt, :], ps[:])

                xg_sb = xg_pool.tile([P, NKT, UNIT], BF16, tag="xg")
                for h in range(NH):
                    for j in range(MSUB):
                        x_sb = chunks[u][h * MSUB + j]
                        for cc in range(HBLK // 4):
                            ps = pst.tile([P, 4, P], F32R, tag="pst")
                            for q in range(4):
                                blk = cc * 4 + q
                                nc.tensor.transpose(
                                    ps[:, q, :],
                                    x_sb[:, blk * P:(blk + 1) * P],
                                    ident_sb[:],
                                )
                            blk0 = h * HBLK + cc * 4
                            evict(
                                xt_sb[:, blk0:blk0 + 4, j * P:(j + 1) * P],
                                ps[:],
                            )
                    for kt in range(NKT):
                        if kt not in gathered and kt_ready_h[kt] <= h:
                            emit_gather(kt, xg_sb=xg_sb)
                            gathered.add(kt)
                del chunks[u]
                # x chunks for u+2 reuse the buffers this unit just read;
                # their triggers become ready as these transposes retire.
                if u + 2 < N_UNITS:
                    chunks[u + 2] = dma_chunks(u + 2)
                if u == 0:
                    for s in range(2, NOS):
                        ensure_wt(s)

                # ---- GEMM for this unit ----
                for s in range(NOS):
                    for t in range(MSUB):
                        ps = psb.tile([P, O_TILE], F32, tag="psb",
                                      name=f"psb{u}_{s}_{t}")
                        for kt in range(NKT):
                            nc.tensor.matmul(
                                ps[:],
                                xg_sb[:, kt, t * P:(t + 1) * P],
                                wt_sb[:, s, kt, :],
                                start=(kt == 0),
                                stop=(kt == NKT - 1),
                            )
                        o_sb = o_pool.tile([P, O_TILE], F32, tag="ob",
                                           name=f"ob{u}_{s}_{t}")
                        evict(o_sb[:], ps[:])
                        ring().dma_start(
                            out[m0 + t * P:m0 + (t + 1) * P,
                                s * O_TILE:(s + 1) * O_TILE],
                            o_sb[:],
                        )
    nc.compile()
    return nc


def _reorder_g(g_all, blocks_per_kt):
    """Order g blocks so kts gatherable from column half 0 come first."""
    order = []
    new_blocks = []
    for pass_h in range(2):
        for kt in range(NKT):
            entries = blocks_per_kt[kt]
            ready = 0 if all(b < HBLK for _, b in entries) else 1
            if ready != pass_h:
                continue
            new_entries = []
            for bi, b in entries:
                new_entries.append((len(order), b))
                order.append(bi)
            new_blocks.append((kt, new_entries))
    new_blocks.sort(key=lambda e: e[0])
    return g_all[order], [e for _, e in new_blocks]


def _get_compiled(idx: np.ndarray):
    key = idx.tobytes()
    if key not in _cache:
        g_all, blocks_per_kt = _build_gather_blocks(idx)
        g_all, blocks_per_kt = _reorder_g(g_all, blocks_per_kt)
        nc = _build_nc(blocks_per_kt, g_all.shape[0])
        _cache[key] = (nc, g_all)
    return _cache[key]


def _run(inputs, trace=False, trace_kwargs=None):
    from concourse.bass_utils import run_bass_kernel_spmd

    x = np.ascontiguousarray(np.asarray(inputs["x"], dtype=np.float32))
    sv = np.asarray(inputs["sparse_values"], dtype=np.float32)
    idx = np.asarray(inputs["sparse_idx"]).astype(np.int64)

    nc, g_all = _get_compiled(idx)

    x2 = x.reshape(M_TOT, N_IN)
    # wt swizzled for per-partition DMA: [k%128, o-slice, k//128, o], bf16
    wtv = np.ascontiguousarray(
        sv.T.reshape(NKT, P, NOS, O_TILE).transpose(1, 2, 0, 3)
    ).astype(ml_dtypes.bfloat16)
    # g swizzled to [n-rel (partition), block, k-rel], bf16 (one-hot: exact)
    g_swz = np.ascontiguousarray(
        g_all.transpose(1, 0, 2)).astype(ml_dtypes.bfloat16)
    in_maps = [
        {
            "x": np.ascontiguousarray(x2[c * M:(c + 1) * M]),
            "wt": wtv,
            "g": g_swz,
            "ident": np.eye(P, dtype=np.float32),
        }
        for c in range(N_CORES)
    ]
    res = run_bass_kernel_spmd(
        nc,
        in_maps,
        core_ids=list(range(N_CORES)),
        trace=trace,
        **(trace_kwargs or {}),
    )
    full = np.concatenate([r["out"] for r in res.results], axis=0)
    return full.reshape(B, S, N_OUT), res


def kernel(**inputs) -> np.ndarray:
    out, _ = _run(inputs)
    return out


# revision 15
# speedup vs baseline: 1.0913x; 1.0179x over previous
"""Trainium2 Bass kernel for structured-sparse matmul.

Computes: out[b,s,o] = sum_k x[b,s,sparse_idx[k]] * sparse_values[o,k]
  x: [4, 2048, 4096] f32, sparse_values: [4096, 1024] f32,
  sparse_idx: [1024] int64 (sorted, unique) -> out [4, 2048, 4096] f32

Strategy (8 NeuronCores, data-parallel over rows m = b*s, M=1024/core):
  Pipelined 256-row units: per unit, PE-transpose x (is_transpose mode,
  f32r 1.5 cy/row), gather n->k with one-hot blocks (bf16), then GEMM in
  bf16 (W host-cast, halving its DMA).  Engine discipline learned from
  the v2 trace:
    - DVE owns every PSUM eviction; the Sync/Scalar queues carry ONLY
      HWDGE DMA triggers.  (Mixing ACT evictions with DMA triggers
      head-of-line blocked the PE for 5-8us per unit, re-tripping the
      HAM clock gate and running whole GEMM units at 1.2 GHz.)
    - x streams as 1MB chunks through an 8-buffer pool: ~2 units of
      lookahead, paced automatically by the pool's WAR dependencies.
    - Warm-up transposes read a memset scratch tile, not a DMA'd one,
      so the PE is busy from ~1.5us and the HAM gate opens before the
      first real transpose.
"""

import sys

if "/opt/trn_rl_repo" not in sys.path:
    sys.path.insert(0, "/opt/trn_rl_repo")

import ml_dtypes
import numpy as np

B, S, N_IN = 4, 2048, 4096
N_OUT, N_SPARSE = 4096, 1024
N_CORES = 8
M_TOT = B * S            # 8192
M = M_TOT // N_CORES     # 1024 rows per core
P = 128
NKT = N_SPARSE // P      # 8 k-tiles
NNB = N_IN // P          # 32 n-blocks
UNIT = 256               # pipeline unit (rows)
N_UNITS = M // UNIT      # 4
MSUB = UNIT // P         # 2 m-subtiles per unit
O_TILE = 512
NOS = N_OUT // O_TILE    # 8 o-slices
NH = 2                   # x column halves per unit
HBLK = NNB // NH         # 16 n-blocks per half
XCW = N_IN // NH         # 2048 cols per x chunk
N_WARM = 104           # bridges PE from ~1.5us to first x chunk (~13us)
N_PAD = 30             # filler matmuls between unit-0 chunk transposes

_cache: dict = {}


def _build_gather_blocks(idx: np.ndarray):
    """Expand sparse_idx into one-hot selection blocks.

    For k-tile kt and n-block b, G[n, krel] = 1 iff idx[kt*128+krel] == b*128+n.
    Returns (g_all [NB,128,128] f32, blocks_per_kt: list of lists of (bi, b)).
    """
    mats = []
    blocks_per_kt = []
    for kt in range(NKT):
        ks = idx[kt * P:(kt + 1) * P]
        bs = sorted(set(int(k) // P for k in ks))
        entries = []
        for b in bs:
            mat = np.zeros((P, P), dtype=np.float32)
            for krel, k in enumerate(ks):
                if int(k) // P == b:
                    mat[int(k) % P, krel] = 1.0
            entries.append((len(mats), b))
            mats.append(mat)
        blocks_per_kt.append(entries)
    return np.stack(mats), blocks_per_kt


def _build_nc(blocks_per_kt, nb_total):
    import concourse.mybir as mybir
    import concourse.tile as tile
    from concourse import bacc

    F32R = mybir.dt.float32r
    F32 = mybir.dt.float32
    BF16 = mybir.dt.bfloat16

    nc = bacc.Bacc("TRN2", target_bir_lowering=False, debug=False)
    x = nc.dram_tensor("x", [M, N_IN], F32R, kind="ExternalInput")
    wt = nc.dram_tensor("wt", [P, NOS, NKT, O_TILE], BF16, kind="ExternalInput")
    g = nc.dram_tensor("g", [P, nb_total, P], BF16, kind="ExternalInput")
    ident = nc.dram_tensor("ident", [P, P], F32R, kind="ExternalInput")
    out = nc.dram_tensor("out", [M, N_OUT], F32, kind="ExternalOutput")

    # earliest column-half after which kt's gather can run
    kt_ready_h = []
    for kt in range(NKT):
        entries = blocks_per_kt[kt]
        kt_ready_h.append(0 if all(b < HBLK for _, b in entries) else 1)

    with tile.TileContext(nc) as tc:
        with (
            tc.tile_pool(name="const", bufs=1) as const_pool,
            tc.tile_pool(name="gpool", bufs=1) as g_pool,
            tc.tile_pool(name="wpool", bufs=1) as wt_pool,
            tc.tile_pool(name="xch", bufs=8) as x_pool,
            tc.tile_pool(name="xtpool", bufs=1) as xt_pool,
            tc.tile_pool(name="xgpool", bufs=2) as xg_pool,
            tc.tile_pool(name="opool", bufs=4) as o_pool,
            tc.tile_pool(name="ps_t", bufs=2, space="PSUM") as pst,
            tc.tile_pool(name="ps_g", bufs=2, space="PSUM") as psg,
            tc.tile_pool(name="ps_b", bufs=4, space="PSUM") as psb,
        ):
            # Warm-up on a memset scratch tile: no DMA dependency, so the
            # PE starts within ~1.5us of kernel start; sized to bridge the
            # ~13us until the first x chunk lands (8.4us ring preamble +
            # 5us transfer), keeping the HAM clock gate open.  (bf16: an
            # f32r memset fails the walrus memset_set_value_type check.)
            scratch = const_pool.tile([P, UNIT], BF16, name="scratch")
            nc.vector.memset(scratch[:], 0.0)
            warm_n = [0]

            def warm(n):
                for _ in range(n):
                    w = warm_n[0]
                    warm_n[0] += 1
                    wps = psg.tile([P, UNIT], F32, tag="psg", name=f"warm{w}")
                    nc.tensor.matmul(
                        wps[:], scratch[:, :P], scratch[:],
                        start=True, stop=True,
                    )

            warm(N_WARM)

            ident_sb = const_pool.tile([P, P], F32R)
            nc.sync.dma_start(ident_sb[:], ident[:])
            g_sb = g_pool.tile([P, nb_total, P], BF16)
            wt_sb = wt_pool.tile([P, NOS, NKT, O_TILE], BF16)
            xt_sb = xt_pool.tile([P, NNB, UNIT], BF16)

            ring_flip = [0]

            def ring():
                ring_flip[0] += 1
                return nc.sync if ring_flip[0] % 2 == 0 else nc.scalar

            def evict(dst, src):
                # DVE owns the PE-critical evictions (xt/xg); GPSIMD has
                # no PSUM port and ACT's queue must not block them.
                nc.vector.tensor_copy(dst, src)

            ev_b = [0]

            def evict_b(dst, src):
                # GEMM output evictions alternate DVE/ACT: ACT's trigger
                # queue only ever waits sub-us here, and this halves the
                # DVE latency pressure on psb-buffer reuse.
                if ev_b[0] % 2 == 0:
                    nc.vector.tensor_copy(dst, src)
                else:
                    nc.scalar.copy(dst, src)
                ev_b[0] += 1

            def dma_chunks(u):
                # 4 x 1MB chunk DMAs for unit u, h0 before h1; the 8-buf
                # pool paces triggers ~2 units ahead of consumption.
                tiles = []
                m0 = u * UNIT
                for h in range(NH):
                    for j in range(MSUB):
                        t_ = x_pool.tile([P, XCW], F32R, tag="xch",
                                         name=f"x{u}_{h}_{j}")
                        ring().dma_start(
                            t_[:],
                            x[m0 + j * P:m0 + (j + 1) * P,
                              h * XCW:(h + 1) * XCW],
                        )
                        tiles.append(t_)
                return tiles

            def transpose_group(u, ci):
                # transpose one 1MB chunk (16 n-blocks) of unit u into xt
                h, j = divmod(ci, MSUB)
                x_sb = chunks[u][ci]
                for cc in range(HBLK // 4):
                    ps = pst.tile([P, 4, P], F32R, tag="pst")
                    for q in range(4):
                        blk = cc * 4 + q
                        nc.tensor.transpose(
                            ps[:, q, :],
                            x_sb[:, blk * P:(blk + 1) * P],
                            ident_sb[:],
                        )
                    blk0 = h * HBLK + cc * 4
                    evict(
                        xt_sb[:, blk0:blk0 + 4, j * P:(j + 1) * P],
                        ps[:],
                    )

            def emit_gather(u, kt, xg):
                entries = blocks_per_kt[kt]
                ps = psg.tile([P, UNIT], F32, tag="psg", name=f"psg{u}_{kt}")
                for i, (bi, b) in enumerate(entries):
                    nc.tensor.matmul(
                        ps[:],
                        g_sb[:, bi, :],
                        xt_sb[:, b, :],
                        start=(i == 0),
                        stop=(i == len(entries) - 1),
                    )
                evict(xg[:, kt, :], ps[:])

            def gemm_group(u, s, t, xg):
                ps = psb.tile([P, O_TILE], F32, tag="psb",
                              name=f"psb{u}_{s}_{t}")
                for kt in range(NKT):
                    nc.tensor.matmul(
                        ps[:],
                        xg[:, kt, t * P:(t + 1) * P],
                        wt_sb[:, s, kt, :],
                        start=(kt == 0),
                        stop=(kt == NKT - 1),
                    )
                o_sb = o_pool.tile([P, O_TILE], F32, tag="ob",
                                   name=f"ob{u}_{s}_{t}")
                evict_b(o_sb[:], ps[:])
                ring().dma_start(
                    out[u * UNIT + t * P:u * UNIT + (t + 1) * P,
                        s * O_TILE:(s + 1) * O_TILE],
                    o_sb[:],
                )

            nb_h = sum(
                len(blocks_per_kt[kt]) for kt in range(NKT)
                if kt_ready_h[kt] == 0
            )
            kts_h0 = [kt for kt in range(NKT) if kt_ready_h[kt] == 0]
            chunks = {0: dma_chunks(0)}
            # Deadline-ordered startup stream: g right after unit-0 x;
            # W0-3 (needed from GEMM(0) s=0 at ~25us); x(1) c0/c1 (needed
            # by the transposes embedded at GEMM(0) s=2/s=4); W4-7; x(1)
            # c2/c3 (embedded at s=5/s=6).
            nc.scalar.dma_start(g_sb[:, :nb_h, :], g[:, :nb_h, :])
            nc.sync.dma_start(g_sb[:, nb_h:, :], g[:, nb_h:, :])
            for s in range(4):
                ring().dma_start(wt_sb[:, s, :, :], wt[:, s, :, :])
            chunks[1] = []
            for ci in range(2):
                t_ = x_pool.tile([P, XCW], F32R, tag="xch", name=f"x1_{ci}")
                ring().dma_start(
                    t_[:], x[UNIT + (ci % MSUB) * P:UNIT + (ci % MSUB + 1) * P,
                             (ci // MSUB) * XCW:(ci // MSUB + 1) * XCW])
                chunks[1].append(t_)
            for s in range(4, NOS):
                ring().dma_start(wt_sb[:, s, :, :], wt[:, s, :, :])
            for ci in range(2, 4):
                t_ = x_pool.tile([P, XCW], F32R, tag="xch", name=f"x1_{ci}")
                ring().dma_start(
                    t_[:], x[UNIT + (ci % MSUB) * P:UNIT + (ci % MSUB + 1) * P,
                             (ci // MSUB) * XCW:(ci // MSUB + 1) * XCW])
                chunks[1].append(t_)

            # ---- unit 0 phase A: chunk-paced, warm filler over DMA waits ----
            xg_tiles = {0: xg_pool.tile([P, NKT, UNIT], BF16, tag="xg")}
            for ci in range(2 * MSUB):
                transpose_group(0, ci)
                if ci == 1:
                    warm(N_PAD)
            warm(4)  # bridge the last ~0.5us until g lands
            for kt in range(NKT):
                emit_gather(0, kt, xg_tiles[0])
            del chunks[0]

            # ---- GEMM(u) with unit-u+1 phase A embedded in the s-loop ----
            embed = {2: 0, 4: 1, 5: 2, 6: 3}
            for u in range(N_UNITS):
                nxt = u + 1 if u + 1 < N_UNITS else None
                if nxt is not None:
                    xg_tiles[nxt] = xg_pool.tile([P, NKT, UNIT], BF16,
                                                 tag="xg")
                for s in range(NOS):
                    for t in range(MSUB):
                        gemm_group(u, s, t, xg_tiles[u])
                    if s == 0 and u + 2 < N_UNITS:
                        chunks[u + 2] = dma_chunks(u + 2)
                    if nxt is not None and s in embed:
                        ci = embed[s]
                        transpose_group(nxt, ci)
                        if ci == 1:
                            for kt in kts_h0:
                                emit_gather(nxt, kt, xg_tiles[nxt])
                if nxt is not None:
                    for kt in range(NKT):
                        if kt_ready_h[kt] != 0:
                            emit_gather(nxt, kt, xg_tiles[nxt])
                    del chunks[nxt]
                del xg_tiles[u]
    nc.compile()
    return nc


def _reorder_g(g_all, blocks_per_kt):
    """Order g blocks so kts gatherable from column half 0 come first."""
    order = []
    new_blocks = []
    for pass_h in range(2):
        for kt in range(NKT):
            entries = blocks_per_kt[kt]
            ready = 0 if all(b < HBLK for _, b in entries) else 1
            if ready != pass_h:
                continue
            new_entries = []
            for bi, b in entries:
                new_entries.append((len(order), b))
                order.append(bi)
            new_blocks.append((kt, new_entries))
    new_blocks.sort(key=lambda e: e[0])
    return g_all[order], [e for _, e in new_blocks]


def _get_compiled(idx: np.ndarray):
    key = idx.tobytes()
    if key not in _cache:
        g_all, blocks_per_kt = _build_gather_blocks(idx)
        g_all, blocks_per_kt = _reorder_g(g_all, blocks_per_kt)
        nc = _build_nc(blocks_per_kt, g_all.shape[0])
        _cache[key] = (nc, g_all)
    return _cache[key]


def _run(inputs, trace=False, trace_kwargs=None):
    from concourse.bass_utils import run_bass_kernel_spmd

    x = np.ascontiguousarray(np.asarray(inputs["x"], dtype=np.float32))
    sv = np.asarray(inputs["sparse_values"], dtype=np.float32)
    idx = np.asarray(inputs["sparse_idx"]).astype(np.int64)

    nc, g_all = _get_compiled(idx)

    x2 = x.reshape(M_TOT, N_IN)
    # wt swizzled for per-partition DMA: [k%128, o-slice, k//128, o], bf16
    wtv = np.ascontiguousarray(
        sv.T.reshape(NKT, P, NOS, O_TILE).transpose(1, 2, 0, 3)
    ).astype(ml_dtypes.bfloat16)
    # g swizzled to [n-rel (partition), block, k-rel], bf16 (one-hot: exact)
    g_swz = np.ascontiguousarray(
        g_all.transpose(1, 0, 2)).astype(ml_dtypes.bfloat16)
    in_maps = [
        {
            "x": np.ascontiguousarray(x2[c * M:(c + 1) * M]),
            "wt": wtv,
            "g": g_swz,
            "ident": np.eye(P, dtype=np.float32),
        }
        for c in range(N_CORES)
    ]
    res = run_bass_kernel_spmd(
        nc,
        in_maps,
        core_ids=list(range(N_CORES)),
        trace=trace,
        **(trace_kwargs or {}),
    )
    full = np.concatenate([r["out"] for r in res.results], axis=0)
    return full.reshape(B, S, N_OUT), res


def kernel(**inputs) -> np.ndarray:
    out, _ = _run(inputs)
    return out
